# revision 3
# baseline (speedup 1.0000x reference)
"""Trainium2 Bass kernel for nn_ComputeDistances (vq_codebook).

dist[k, m] = || X @ (M[:, m] - c_k) ||_2,  X:[4096,512], M:[512,4096], C:[2048,512]

Reformulated via the Gram matrix G = X^T X (512x512):
    dist^2[k, m] = m^T G m  -  2 c_k^T G m  +  c_k^T G c_k
which drops total FLOPs from ~95G to ~14G.

Sharding: 8 cores as a 2(K) x 4(m) grid; each core computes its
[1024, 1024] output slab independently (no collectives).

Per-core device pipeline (all matmuls in fp32r: full PE rate, ~14x better
precision than bf16; fp32r operands must be DMA-produced, so intermediates
are cast via SBUF->SBUF DMA):
  A: GXX = X^T X                      (fp32 PSUM, 128 MMs)
  B: H   = GXX @ Ms                   (32 MMs)   -> h_r (fp32r)
     GC2 = GXX @ (-2 CTs)             (32 MMs)
     sqXM[m] = sum_d (H .* Ms)        (ones-matmul rows, fp32)
     sqXC[k] = 0.25 * sum_d (GC2 .* -2CTs)  (ones-matmul cols, fp32)
  C: G2  = (-2 CTs)^T @ H             (64 MMs)  == -2 * XC^T XM part
     out = sqrt(G2 + sqXM_bcast + sqXC_col)    (DVE add + ACT sqrt w/ bias)
"""

import os
import numpy as np

N, D, M_COLS, K = 4096, 512, 4096, 2048
N_CORES = 8
KC, MC = 2, 4  # core grid: K-split x M-split
K_LOC, M_LOC = K // KC, M_COLS // MC  # 1024, 1024

P = 128
NT = N // P        # 32 X row-tiles
DC = D // P        # 4 contraction chunks over D
MS = M_LOC // 512  # 2 m-slices of 512
KT = K_LOC // P    # 8 k-tiles

_compiled = {}


def _build_nc():
    import concourse.mybir as mybir
    import concourse.tile as tile
    from concourse import bacc

    f32 = mybir.dt.float32
    f32r = mybir.dt.float32r
    ADD = mybir.AluOpType.add
    MULT = mybir.AluOpType.mult

    nc = bacc.Bacc("TRN2", target_bir_lowering=False, debug=False)

    x_d = nc.dram_tensor("x", [N, D], f32, kind="ExternalInput")
    m_d = nc.dram_tensor("ms", [D, M_LOC], f32, kind="ExternalInput")
    c_d = nc.dram_tensor("cts2", [D, K_LOC], f32, kind="ExternalInput")  # -2*C_s^T
    o_d = nc.dram_tensor("out", [K_LOC, M_LOC], f32, kind="ExternalOutput")

    with tile.TileContext(nc) as tc:
        with (
            tc.tile_pool(name="xp", bufs=1) as xp,
            tc.tile_pool(name="inp", bufs=1) as inp,
            tc.tile_pool(name="res", bufs=1) as res,
            tc.tile_pool(name="wk", bufs=2) as wk,
            tc.tile_pool(name="op", bufs=3) as op,
            tc.tile_pool(name="psA", bufs=2, space="PSUM") as psA,
            tc.tile_pool(name="psG", bufs=1, space="PSUM") as psG,
            tc.tile_pool(name="psS", bufs=1, space="PSUM") as psS,
        ):
            # ---- input loads ----
            xt = []
            for i in range(NT):
                t = xp.tile([P, D], f32r, tag=f"x{i}", name=f"x{i}")
                nc.sync.dma_start(t[:], x_d.ap()[i * P : (i + 1) * P, :].bitcast(f32r))
                xt.append(t)
            ms_r, ct_r = [], []
            for c in range(DC):
                t = inp.tile([P, M_LOC], f32r, tag=f"ms{c}", name=f"ms{c}")
                nc.sync.dma_start(t[:], m_d.ap()[c * P : (c + 1) * P, :].bitcast(f32r))
                ms_r.append(t)
                t = inp.tile([P, K_LOC], f32r, tag=f"ct{c}", name=f"ct{c}")
                nc.sync.dma_start(t[:], c_d.ap()[c * P : (c + 1) * P, :].bitcast(f32r))
                ct_r.append(t)

            ones = res.tile([P, 1], f32, tag="ones")
            nc.vector.memset(ones[:], 1.0)
            quarter = res.tile([P, 1], f32, tag="quarter")
            nc.vector.memset(quarter[:], 0.25)

            # resident intermediates
            gxx_r = [res.tile([P, D], f32r, tag=f"gxxr{t}", name=f"gxxr{t}") for t in range(DC)]
            h_r = [res.tile([P, M_LOC], f32r, tag=f"hr{t}", name=f"hr{t}") for t in range(DC)]
            q_t = [res.tile([P, K_LOC], f32, tag=f"q{t}", name=f"q{t}") for t in range(DC)]
            sq_row_sb = res.tile([1, M_LOC], f32, tag="sqrow_sb")
            sqxm_b = res.tile([P, M_LOC], f32, tag="sqxm_b")
            sqxc_sb = res.tile([P, KT], f32, tag="sqxc_sb")

            # ---- stage A: GXX = X^T X ----
            for t in range(DC):
                pg = psG.tile([P, D], mybir.dt.float32, tag="gxx")
                for i in range(NT):
                    nc.tensor.matmul(
                        pg[:],
                        xt[i][:, t * P : (t + 1) * P],
                        xt[i][:],
                        start=(i == 0),
                        stop=(i == NT - 1),
                    )
                gf = wk.tile([P, D], f32, tag="gxxf")
                nc.vector.tensor_copy(gf[:], pg[:])
                nc.sync.dma_start(gxx_r[t][:], gf[:].bitcast(f32r))

            # ---- stage B: H = GXX @ Ms ; sqXM ----
            sq_row = psS.tile([1, M_LOC], mybir.dt.float32, tag="sqrow")
            for t in range(DC):
                hf = wk.tile([P, M_LOC], f32, tag="hf")
                for s in range(MS):
                    ph = psA.tile([P, 512], mybir.dt.float32, tag="ph")
                    for c in range(DC):
                        nc.tensor.matmul(
                            ph[:],
                            gxx_r[c][:, t * P : (t + 1) * P],
                            ms_r[c][:, s * 512 : (s + 1) * 512],
                            start=(c == 0),
                            stop=(c == DC - 1),
                        )
                    nc.vector.tensor_copy(hf[:, s * 512 : (s + 1) * 512], ph[:])
                nc.sync.dma_start(h_r[t][:], hf[:].bitcast(f32r))
                p = wk.tile([P, M_LOC], f32, tag="p")
                nc.vector.tensor_tensor(p[:], hf[:], ms_r[t][:].bitcast(f32), MULT)
                for s in range(MS):
                    nc.tensor.matmul(
                        sq_row[:, s * 512 : (s + 1) * 512],
                        ones[:],
                        p[:, s * 512 : (s + 1) * 512],
                        start=(t == 0),
                        stop=(t == DC - 1),
                    )

            # ---- stage B2: GC2 = GXX @ (-2 CTs) ; q = GC2 .* (-2CTs) ----
            for t in range(DC):
                gcf = wk.tile([P, K_LOC], f32, tag="gcf")
                for s in range(K_LOC // 512):
                    ph = psA.tile([P, 512], mybir.dt.float32, tag="ph")
                    for c in range(DC):
                        nc.tensor.matmul(
                            ph[:],
                            gxx_r[c][:, t * P : (t + 1) * P],
                            ct_r[c][:, s * 512 : (s + 1) * 512],
                            start=(c == 0),
                            stop=(c == DC - 1),
                        )
                    nc.vector.tensor_copy(gcf[:, s * 512 : (s + 1) * 512], ph[:])
                nc.vector.tensor_tensor(q_t[t][:], gcf[:], ct_r[t][:].bitcast(f32), MULT)

            # sqXC[k] = 0.25 * sum_d q  -> column form [128, 1] per k-tile
            for kt in range(KT):
                pc = psS.tile([P, 1], mybir.dt.float32, tag="sqcol")
                for c in range(DC):
                    nc.tensor.matmul(
                        pc[:],
                        q_t[c][:, kt * P : (kt + 1) * P],
                        quarter[:],
                        start=(c == 0),
                        stop=(c == DC - 1),
                    )
                nc.vector.tensor_copy(sqxc_sb[:, kt : kt + 1], pc[:])

            # sqXM row -> broadcast to 128 partitions (GPSIMD custom inst)
            nc.vector.tensor_copy(sq_row_sb[:], sq_row[:])
            nc.gpsimd.partition_broadcast(sqxm_b[:], sq_row_sb[:1, :])

            # ---- stage C: G2 = (-2CTs)^T @ H ; combine ; sqrt ----
            for kt in range(KT):
                for s in range(MS):
                    pgc = psA.tile([P, 512], mybir.dt.float32, tag="pg")
                    for c in range(DC):
                        nc.tensor.matmul(
                            pgc[:],
                            ct_r[c][:, kt * P : (kt + 1) * P],
                            h_r[c][:, s * 512 : (s + 1) * 512],
                            start=(c == 0),
                            stop=(c == DC - 1),
                        )
                    t1 = wk.tile([P, 512], f32, tag="t1")
                    nc.vector.tensor_tensor(
                        t1[:], pgc[:], sqxm_b[:, s * 512 : (s + 1) * 512], ADD
                    )
                    ob = op.tile([P, 512], f32, tag="ob")
                    nc.scalar.activation(
                        ob[:],
                        t1[:],
                        mybir.ActivationFunctionType.Sqrt,
                        bias=sqxc_sb[:, kt : kt + 1],
                    )
                    nc.sync.dma_start(
                        o_d.ap()[kt * P : (kt + 1) * P, s * 512 : (s + 1) * 512],
                        ob[:],
                    )

    nc.compile()
    return nc


def _get_nc():
    if "nc" not in _compiled:
        _compiled["nc"] = _build_nc()
    return _compiled["nc"]


def kernel(in_activations, M, centroids):
    from concourse import bass_utils

    X = np.ascontiguousarray(np.asarray(in_activations, dtype=np.float32))
    Mf = np.asarray(M, dtype=np.float32)
    C = np.asarray(centroids, dtype=np.float32)

    nc = _get_nc()

    in_maps = []
    for core in range(N_CORES):
        kc, mc = divmod(core, MC)
        ms = np.ascontiguousarray(Mf[:, mc * M_LOC : (mc + 1) * M_LOC])
        cts2 = np.ascontiguousarray(
            (-2.0 * C[kc * K_LOC : (kc + 1) * K_LOC, :].T).astype(np.float32)
        )
        in_maps.append({"x": X, "ms": ms, "cts2": cts2})

    res = bass_utils.run_bass_kernel_spmd(
        nc,
        in_maps,
        core_ids=list(range(N_CORES)),
        trace=bool(int(os.environ.get("KERNEL_TRACE", "0"))),
    )
    if res.exec_time_ns is not None:
        print(f"HW exec time: {res.exec_time_ns} ns")
        _compiled["exec_time_ns"] = res.exec_time_ns

    out = np.empty((K, M_COLS), dtype=np.float32)
    for core in range(N_CORES):
        kc, mc = divmod(core, MC)
        out[kc * K_LOC : (kc + 1) * K_LOC, mc * M_LOC : (mc + 1) * M_LOC] = res.results[
            core
        ]["out"]
    return out


# revision 5
# speedup vs baseline: 1.0254x; 1.0254x over previous
"""Trainium2 Bass kernel for nn_ComputeDistances (vq_codebook).

dist[k, m] = || X @ (M[:, m] - c_k) ||_2,  X:[4096,512], M:[512,4096], C:[2048,512]

Reformulated via the Gram matrix G = X^T X (512x512):
    dist^2[k, m] = m^T G m  -  2 c_k^T G m  +  c_k^T G c_k
which drops total FLOPs from ~95G to ~14G.

Sharding: 8 cores as a 2(K) x 4(m) grid; each core computes its
[1024, 1024] output slab independently (no collectives).

All matmuls run in fp32r (full PE rate for free-dim>=256, ~11 mantissa
bits). fp32r operands must be DMA-produced, so engine-produced
intermediates are cast via SBUF->SBUF DMA.

Optimizations over v1:
  - warmup: ~56 tiny bf16 matmuls on zero tiles during the input-DMA wait
    so the PE HAM clock-gate reaches 2.4 GHz before real work starts
  - GXX symmetry: only upper-triangular 128-blocks are computed; lower
    blocks are PE-transposed copies (saves 37.5% of stage-A matmul work)
  - sq-norm reductions (ones-matmuls) in fp32r instead of fp32
    (fp32 matmul = 2 half-rate passes; fp32r single full-rate pass)
"""

import os
import numpy as np

N, D, M_COLS, K = 4096, 512, 4096, 2048
N_CORES = 8
KC, MC = 2, 4  # core grid: K-split x M-split
K_LOC, M_LOC = K // KC, M_COLS // MC  # 1024, 1024

P = 128
NT = N // P        # 32 X row-tiles
DC = D // P        # 4 contraction chunks over D
MS = M_LOC // 512  # 2 m-slices of 512
KT = K_LOC // P    # 8 k-tiles
WARM_MMS = 56

_compiled = {}


def _build_nc():
    import concourse.mybir as mybir
    import concourse.tile as tile
    from concourse import bacc
    from concourse.masks import make_identity

    f32 = mybir.dt.float32
    f32r = mybir.dt.float32r
    bf16 = mybir.dt.bfloat16
    ADD = mybir.AluOpType.add
    MULT = mybir.AluOpType.mult

    nc = bacc.Bacc("TRN2", target_bir_lowering=False, debug=False)

    x_d = nc.dram_tensor("x", [N, D], f32, kind="ExternalInput")
    m_d = nc.dram_tensor("ms", [D, M_LOC], f32, kind="ExternalInput")
    c_d = nc.dram_tensor("cts2", [D, K_LOC], f32, kind="ExternalInput")  # -2*C_s^T
    cst_d = nc.dram_tensor("cst", [P, 130], f32, kind="ExternalInput")  # ones block | 0.25 | 0
    o_d = nc.dram_tensor("out", [K_LOC, M_LOC], f32, kind="ExternalOutput")

    with tile.TileContext(nc) as tc:
        with (
            tc.tile_pool(name="xp", bufs=1) as xp,
            tc.tile_pool(name="inp", bufs=1) as inp,
            tc.tile_pool(name="res", bufs=1) as res,
            tc.tile_pool(name="wk", bufs=2) as wk,
            tc.tile_pool(name="op", bufs=3) as op,
            tc.tile_pool(name="psA", bufs=2, space="PSUM") as psA,
            tc.tile_pool(name="psG", bufs=1, space="PSUM") as psG,
            tc.tile_pool(name="psS", bufs=1, space="PSUM") as psS,
        ):
            # ---- PE warmup: tiny bf16 matmuls on zero tiles (no input deps) ----
            wl = res.tile([P, 1], bf16, tag="wl")
            wz = res.tile([P, 64], bf16, tag="wz")
            nc.vector.memset(wl[:], 0.0)
            nc.vector.memset(wz[:], 0.0)
            wps = psG.tile([1, 64], mybir.dt.float32, tag="gxx")
            for _ in range(WARM_MMS):
                nc.tensor.matmul(wps[:], wl[:], wz[:], start=True, stop=True)

            # ---- input loads ----
            xt = []
            for i in range(NT):
                t = xp.tile([P, D], f32r, tag=f"x{i}", name=f"x{i}")
                nc.sync.dma_start(t[:], x_d.ap()[i * P : (i + 1) * P, :].bitcast(f32r))
                xt.append(t)
            ms_r, ct_r = [], []
            for c in range(DC):
                t = inp.tile([P, M_LOC], f32r, tag=f"ms{c}", name=f"ms{c}")
                nc.sync.dma_start(t[:], m_d.ap()[c * P : (c + 1) * P, :].bitcast(f32r))
                ms_r.append(t)
                t = inp.tile([P, K_LOC], f32r, tag=f"ct{c}", name=f"ct{c}")
                nc.sync.dma_start(t[:], c_d.ap()[c * P : (c + 1) * P, :].bitcast(f32r))
                ct_r.append(t)
            cst_r = res.tile([P, 130], f32r, tag="cst")
            nc.sync.dma_start(cst_r[:], cst_d.ap().bitcast(f32r))
            ones_blk_r = cst_r[:, 0:P]      # [128,128] all-ones stationary
            quarter2_r = cst_r[:, P : P + 2]  # [128,2]: col0=0.25, col1=0

            ident = res.tile([P, P], f32, tag="ident")
            make_identity(nc, ident[:])

            # resident intermediates
            gxx_f32 = [
                res.tile([P, D], f32, tag=f"gxxf{t}", name=f"gxxf{t}")
                for t in range(DC)
            ]
            gxx_r = [
                res.tile([P, D], f32r, tag=f"gxxr{t}", name=f"gxxr{t}")
                for t in range(DC)
            ]
            h_r = [
                res.tile([P, M_LOC], f32r, tag=f"hr{t}", name=f"hr{t}")
                for t in range(DC)
            ]
            q_r = [
                res.tile([P, K_LOC], f32r, tag=f"qr{t}", name=f"qr{t}")
                for t in range(DC)
            ]
            sqxm_b = res.tile([P, M_LOC], f32, tag="sqxm_b")
            sqxc_sb = res.tile([P, KT], f32, tag="sqxc_sb")

            # ---- stage A: GXX = X^T X (upper-triangular blocks + mirror) ----
            for t in range(DC):
                width = D - t * P
                pg = psG.tile([P, D], mybir.dt.float32, tag="gxx")
                for i in range(NT):
                    nc.tensor.matmul(
                        pg[:, :width],
                        xt[i][:, t * P : (t + 1) * P],
                        xt[i][:, t * P :],
                        start=(i == 0),
                        stop=(i == NT - 1),
                    )
                nc.vector.tensor_copy(gxx_f32[t][:, t * P :], pg[:, :width])
                # mirror block (t, c>t) into row-blocks below: gxx_f32[c][:, t-block]
                for c in range(t + 1, DC):
                    tp = psA.tile([P, 512], mybir.dt.float32, tag="ph")
                    nc.tensor.transpose(
                        tp[:, :P], gxx_f32[t][:, c * P : (c + 1) * P], ident[:]
                    )
                    nc.vector.tensor_copy(gxx_f32[c][:, t * P : (t + 1) * P], tp[:, :P])
                if t == DC - 1:
                    for c in range(DC):
                        nc.sync.dma_start(gxx_r[c][:], gxx_f32[c][:].bitcast(f32r))

            # ---- stage B: H = GXX @ Ms ; sqXM via fp32r ones-block matmul ----
            # ones-block stationary [128,128] => every PSUM partition gets the
            # same column sum, i.e. sqXM arrives already partition-broadcast.
            sqm = [
                psS.tile([P, 512], mybir.dt.float32, tag=f"sqm{s}", name=f"sqm{s}")
                for s in range(MS)
            ]
            for t in range(DC):
                hf = wk.tile([P, M_LOC], f32, tag="hf")
                for s in range(MS):
                    ph = psA.tile([P, 512], mybir.dt.float32, tag="ph")
                    for c in range(DC):
                        nc.tensor.matmul(
                            ph[:],
                            gxx_r[c][:, t * P : (t + 1) * P],
                            ms_r[c][:, s * 512 : (s + 1) * 512],
                            start=(c == 0),
                            stop=(c == DC - 1),
                        )
                    nc.vector.tensor_copy(hf[:, s * 512 : (s + 1) * 512], ph[:])
                nc.sync.dma_start(h_r[t][:], hf[:].bitcast(f32r))
                p = wk.tile([P, M_LOC], f32, tag="p")
                nc.vector.tensor_tensor(p[:], hf[:], ms_r[t][:].bitcast(f32), MULT)
                pr = wk.tile([P, M_LOC], f32r, tag="pr")
                nc.sync.dma_start(pr[:], p[:].bitcast(f32r))
                for s in range(MS):
                    nc.tensor.matmul(
                        sqm[s][:],
                        ones_blk_r,
                        pr[:, s * 512 : (s + 1) * 512],
                        start=(t == 0),
                        stop=(t == DC - 1),
                    )
                if t == DC - 1:
                    for s in range(MS):
                        nc.vector.tensor_copy(
                            sqxm_b[:, s * 512 : (s + 1) * 512], sqm[s][:]
                        )

            # ---- stage B2: GC2 = GXX @ (-2 CTs) ; q = GC2 .* (-2CTs) ----
            for t in range(DC):
                gcf = wk.tile([P, K_LOC], f32, tag="gcf")
                for s in range(K_LOC // 512):
                    ph = psA.tile([P, 512], mybir.dt.float32, tag="ph")
                    for c in range(DC):
                        nc.tensor.matmul(
                            ph[:],
                            gxx_r[c][:, t * P : (t + 1) * P],
                            ct_r[c][:, s * 512 : (s + 1) * 512],
                            start=(c == 0),
                            stop=(c == DC - 1),
                        )
                    nc.vector.tensor_copy(gcf[:, s * 512 : (s + 1) * 512], ph[:])
                qf = wk.tile([P, K_LOC], f32, tag="qf")
                nc.vector.tensor_tensor(qf[:], gcf[:], ct_r[t][:].bitcast(f32), MULT)
                nc.sync.dma_start(q_r[t][:], qf[:].bitcast(f32r))

            # sqXC[k] = 0.25 * sum_d q  -> column form [128, 1] per k-tile
            for kt in range(KT):
                pc = psS.tile([P, 2], mybir.dt.float32, tag="sqcol")
                for c in range(DC):
                    nc.tensor.matmul(
                        pc[:],
                        q_r[c][:, kt * P : (kt + 1) * P],
                        quarter2_r,
                        start=(c == 0),
                        stop=(c == DC - 1),
                    )
                nc.vector.tensor_copy(sqxc_sb[:, kt : kt + 1], pc[:, 0:1])

            # ---- stage C: G2 = (-2CTs)^T @ H ; combine ; sqrt ----
            for kt in range(KT):
                for s in range(MS):
                    pgc = psA.tile([P, 512], mybir.dt.float32, tag="pg")
                    for c in range(DC):
                        nc.tensor.matmul(
                            pgc[:],
                            ct_r[c][:, kt * P : (kt + 1) * P],
                            h_r[c][:, s * 512 : (s + 1) * 512],
                            start=(c == 0),
                            stop=(c == DC - 1),
                        )
                    t1 = wk.tile([P, 512], f32, tag="t1")
                    nc.vector.tensor_tensor(
                        t1[:], pgc[:], sqxm_b[:, s * 512 : (s + 1) * 512], ADD
                    )
                    ob = op.tile([P, 512], f32, tag="ob")
                    nc.scalar.activation(
                        ob[:],
                        t1[:],
                        mybir.ActivationFunctionType.Sqrt,
                        bias=sqxc_sb[:, kt : kt + 1],
                    )
                    nc.sync.dma_start(
                        o_d.ap()[kt * P : (kt + 1) * P, s * 512 : (s + 1) * 512],
                        ob[:],
                    )

    nc.compile()
    return nc


def _get_nc():
    if "nc" not in _compiled:
        _compiled["nc"] = _build_nc()
    return _compiled["nc"]


def kernel(in_activations, M, centroids):
    from concourse import bass_utils

    X = np.ascontiguousarray(np.asarray(in_activations, dtype=np.float32))
    Mf = np.asarray(M, dtype=np.float32)
    C = np.asarray(centroids, dtype=np.float32)

    nc = _get_nc()

    cst = np.zeros((P, 130), dtype=np.float32)
    cst[:, :P] = 1.0
    cst[:, P] = 0.25

    in_maps = []
    for core in range(N_CORES):
        kc, mc = divmod(core, MC)
        ms = np.ascontiguousarray(Mf[:, mc * M_LOC : (mc + 1) * M_LOC])
        cts2 = np.ascontiguousarray(
            (-2.0 * C[kc * K_LOC : (kc + 1) * K_LOC, :].T).astype(np.float32)
        )
        in_maps.append({"x": X, "ms": ms, "cts2": cts2, "cst": cst})

    res = bass_utils.run_bass_kernel_spmd(
        nc,
        in_maps,
        core_ids=list(range(N_CORES)),
        trace=bool(int(os.environ.get("KERNEL_TRACE", "0"))),
    )
    if res.exec_time_ns is not None:
        print(f"HW exec time: {res.exec_time_ns} ns")
        _compiled["exec_time_ns"] = res.exec_time_ns

    out = np.empty((K, M_COLS), dtype=np.float32)
    for core in range(N_CORES):
        kc, mc = divmod(core, MC)
        out[kc * K_LOC : (kc + 1) * K_LOC, mc * M_LOC : (mc + 1) * M_LOC] = res.results[
            core
        ]["out"]
    return out


# revision 7
# speedup vs baseline: 1.0757x; 1.0491x over previous
"""Trainium2 Bass kernel for nn_ComputeDistances (vq_codebook).

dist[k, m] = || X @ (M[:, m] - c_k) ||_2,  X:[4096,512], M:[512,4096], C:[2048,512]

Reformulated via the Gram matrix G = X^T X (512x512):
    dist^2[k, m] = m^T G m  -  2 c_k^T G m  +  c_k^T G c_k
which drops total FLOPs from ~95G to ~14G.

Sharding: 8 cores as a 2(K) x 4(m) grid; each core computes its
[1024, 1024] output slab independently (no collectives).

All matmuls run in fp32r (full PE rate for free-dim>=256, ~11 mantissa
bits). fp32r operands must be DMA-produced, so engine-produced
intermediates are cast via SBUF->SBUF DMA.

Optimizations over v1:
  - warmup: ~56 tiny bf16 matmuls on zero tiles during the input-DMA wait
    so the PE HAM clock-gate reaches 2.4 GHz before real work starts
  - GXX symmetry: only upper-triangular 128-blocks are computed; lower
    blocks are PE-transposed copies (saves 37.5% of stage-A matmul work)
  - sq-norm reductions (ones-matmuls) in fp32r instead of fp32
    (fp32 matmul = 2 half-rate passes; fp32r single full-rate pass)
"""

import os
import numpy as np

N, D, M_COLS, K = 4096, 512, 4096, 2048
N_CORES = 8
KC, MC = 2, 4  # core grid: K-split x M-split
K_LOC, M_LOC = K // KC, M_COLS // MC  # 1024, 1024

P = 128
NT = N // P        # 32 X row-tiles
DC = D // P        # 4 contraction chunks over D
MS = M_LOC // 512  # 2 m-slices of 512
KT = K_LOC // P    # 8 k-tiles
WARM_MMS = 56

_compiled = {}


def _build_nc():
    import concourse.mybir as mybir
    import concourse.tile as tile
    from concourse import bacc
    from concourse.masks import make_identity

    f32 = mybir.dt.float32
    f32r = mybir.dt.float32r
    bf16 = mybir.dt.bfloat16
    ADD = mybir.AluOpType.add
    MULT = mybir.AluOpType.mult

    nc = bacc.Bacc("TRN2", target_bir_lowering=False, debug=False)

    x_d = nc.dram_tensor("x", [N, D], f32, kind="ExternalInput")
    m_d = nc.dram_tensor("ms", [D, M_LOC], f32, kind="ExternalInput")
    c_d = nc.dram_tensor("cts2", [D, K_LOC], f32, kind="ExternalInput")  # -2*C_s^T
    cst_d = nc.dram_tensor("cst", [P, P], f32, kind="ExternalInput")  # all-ones block
    o_d = nc.dram_tensor("out", [K_LOC, M_LOC], f32, kind="ExternalOutput")

    with tile.TileContext(nc) as tc:
        with (
            tc.tile_pool(name="xp", bufs=1) as xp,
            tc.tile_pool(name="inp", bufs=1) as inp,
            tc.tile_pool(name="res", bufs=1) as res,
            tc.tile_pool(name="wk", bufs=2) as wk,
            tc.tile_pool(name="op", bufs=3) as op,
            tc.tile_pool(name="psA", bufs=3, space="PSUM") as psA,
            tc.tile_pool(name="psG", bufs=1, space="PSUM") as psG,
            tc.tile_pool(name="psS", bufs=1, space="PSUM") as psS,
        ):
            # ---- PE warmup: tiny bf16 matmuls on zero tiles (no input deps) ----
            wl = res.tile([P, 1], bf16, tag="wl")
            wz = res.tile([P, 64], bf16, tag="wz")
            nc.vector.memset(wl[:], 0.0)
            nc.vector.memset(wz[:], 0.0)
            wps = psG.tile([1, 64], mybir.dt.float32, tag="gxx")
            for _ in range(WARM_MMS):
                nc.tensor.matmul(wps[:], wl[:], wz[:], start=True, stop=True)

            # ---- input loads ----
            xt = []
            dma_engs = [nc.sync, nc.scalar]
            for i in range(NT):
                t = xp.tile([P, D], f32r, tag=f"x{i}", name=f"x{i}")
                dma_engs[i % 2].dma_start(
                    t[:], x_d.ap()[i * P : (i + 1) * P, :].bitcast(f32r)
                )
                xt.append(t)
            ms_r, ct_r = [], []
            for c in range(DC):
                t = inp.tile([P, M_LOC], f32r, tag=f"ms{c}", name=f"ms{c}")
                nc.scalar.dma_start(t[:], m_d.ap()[c * P : (c + 1) * P, :].bitcast(f32r))
                ms_r.append(t)
                t = inp.tile([P, K_LOC], f32r, tag=f"ct{c}", name=f"ct{c}")
                nc.gpsimd.dma_start(t[:], c_d.ap()[c * P : (c + 1) * P, :].bitcast(f32r))
                ct_r.append(t)
            cst_r = res.tile([P, P], f32r, tag="cst")
            nc.sync.dma_start(cst_r[:], cst_d.ap().bitcast(f32r))
            ones_blk_r = cst_r[:, 0:P]  # [128,128] all-ones stationary

            ident = res.tile([P, P], f32, tag="ident")
            make_identity(nc, ident[:])

            # resident intermediates
            gxx_f32 = [
                res.tile([P, D], f32, tag=f"gxxf{t}", name=f"gxxf{t}")
                for t in range(DC)
            ]
            gxx_r = [
                res.tile([P, D], f32r, tag=f"gxxr{t}", name=f"gxxr{t}")
                for t in range(DC)
            ]
            h_r = [
                res.tile([P, M_LOC], f32r, tag=f"hr{t}", name=f"hr{t}")
                for t in range(DC)
            ]
            sqxm_b = res.tile([P, M_LOC], f32, tag="sqxm_b")
            sqxc_row = res.tile([P, K_LOC], f32, tag="sqxc_row")
            sqxc_sb = res.tile([P, KT], f32, tag="sqxc_sb")

            # ---- stage A: GXX = X^T X (upper-triangular blocks + mirror) ----
            for t in range(DC):
                width = D - t * P
                pg = psG.tile([P, D], mybir.dt.float32, tag="gxx")
                for i in range(NT):
                    nc.tensor.matmul(
                        pg[:, :width],
                        xt[i][:, t * P : (t + 1) * P],
                        xt[i][:, t * P :],
                        start=(i == 0),
                        stop=(i == NT - 1),
                    )
                nc.vector.tensor_copy(gxx_f32[t][:, t * P :], pg[:, :width])
                # mirror block (t, c>t) into row-blocks below: gxx_f32[c][:, t-block]
                for c in range(t + 1, DC):
                    tp = psA.tile([P, 512], mybir.dt.float32, tag="ph")
                    nc.tensor.transpose(
                        tp[:, :P], gxx_f32[t][:, c * P : (c + 1) * P], ident[:]
                    )
                    nc.vector.tensor_copy(gxx_f32[c][:, t * P : (t + 1) * P], tp[:, :P])
                if t == DC - 1:
                    for c in range(DC):
                        nc.sync.dma_start(gxx_r[c][:], gxx_f32[c][:].bitcast(f32r))

            # ---- stage B: H = GXX @ Ms ; sqXM via fp32r ones-block matmul ----
            # ones-block stationary [128,128] => every PSUM partition gets the
            # same column sum, i.e. sqXM arrives already partition-broadcast.
            sqm = [
                psS.tile([P, 512], mybir.dt.float32, tag=f"sqm{s}", name=f"sqm{s}")
                for s in range(MS)
            ]
            for t in range(DC):
                hf = wk.tile([P, M_LOC], f32, tag="hf")
                for s in range(MS):
                    ph = psA.tile([P, 512], mybir.dt.float32, tag="ph")
                    for c in range(DC):
                        nc.tensor.matmul(
                            ph[:],
                            gxx_r[c][:, t * P : (t + 1) * P],
                            ms_r[c][:, s * 512 : (s + 1) * 512],
                            start=(c == 0),
                            stop=(c == DC - 1),
                        )
                    nc.vector.tensor_copy(hf[:, s * 512 : (s + 1) * 512], ph[:])
                nc.scalar.dma_start(h_r[t][:], hf[:].bitcast(f32r))
                p = wk.tile([P, M_LOC], f32, tag="p")
                nc.vector.tensor_tensor(p[:], hf[:], ms_r[t][:].bitcast(f32), MULT)
                pr = wk.tile([P, M_LOC], f32r, tag="pr")
                nc.gpsimd.dma_start(pr[:], p[:].bitcast(f32r))
                for s in range(MS):
                    nc.tensor.matmul(
                        sqm[s][:],
                        ones_blk_r,
                        pr[:, s * 512 : (s + 1) * 512],
                        start=(t == 0),
                        stop=(t == DC - 1),
                    )
                if t == DC - 1:
                    for s in range(MS):
                        nc.vector.tensor_copy(
                            sqxm_b[:, s * 512 : (s + 1) * 512], sqm[s][:]
                        )

            # ---- stage B2: GC2 = GXX @ (-2 CTs) ; q = GC2 .* (-2CTs) ----
            sqc = [
                psS.tile([P, 512], mybir.dt.float32, tag=f"sqc{s}", name=f"sqc{s}")
                for s in range(K_LOC // 512)
            ]
            for t in range(DC):
                gcf = wk.tile([P, K_LOC], f32, tag="gcf")
                for s in range(K_LOC // 512):
                    ph = psA.tile([P, 512], mybir.dt.float32, tag="ph")
                    for c in range(DC):
                        nc.tensor.matmul(
                            ph[:],
                            gxx_r[c][:, t * P : (t + 1) * P],
                            ct_r[c][:, s * 512 : (s + 1) * 512],
                            start=(c == 0),
                            stop=(c == DC - 1),
                        )
                    nc.vector.tensor_copy(gcf[:, s * 512 : (s + 1) * 512], ph[:])
                qf = wk.tile([P, K_LOC], f32, tag="qf")
                nc.vector.tensor_tensor(qf[:], gcf[:], ct_r[t][:].bitcast(f32), MULT)
                qr = wk.tile([P, K_LOC], f32r, tag="qr")
                nc.scalar.dma_start(qr[:], qf[:].bitcast(f32r))
                for s in range(K_LOC // 512):
                    nc.tensor.matmul(
                        sqc[s][:],
                        ones_blk_r,
                        qr[:, s * 512 : (s + 1) * 512],
                        start=(t == 0),
                        stop=(t == DC - 1),
                    )
                if t == DC - 1:
                    for s in range(K_LOC // 512):
                        nc.vector.tensor_copy(
                            sqxc_row[:, s * 512 : (s + 1) * 512], sqc[s][:]
                        )
                    # extract column form: transpose each replicated-row block,
                    # then column 0 holds sqxc for that k-tile (scaled by 0.25
                    # to undo the (-2)^2 from the pre-scaled centroids)
                    for kt in range(KT):
                        tp = psA.tile([P, 512], mybir.dt.float32, tag="ph")
                        nc.tensor.transpose(
                            tp[:, :P],
                            sqxc_row[:, kt * P : (kt + 1) * P],
                            ident[:],
                        )
                        nc.vector.tensor_scalar_mul(
                            sqxc_sb[:, kt : kt + 1], tp[:, 0:1], 0.25
                        )

            # ---- stage C: G2 = (-2CTs)^T @ H ; combine ; sqrt ----
            for kt in range(KT):
                for s in range(MS):
                    pgc = psA.tile([P, 512], mybir.dt.float32, tag="ph")
                    for c in range(DC):
                        nc.tensor.matmul(
                            pgc[:],
                            ct_r[c][:, kt * P : (kt + 1) * P],
                            h_r[c][:, s * 512 : (s + 1) * 512],
                            start=(c == 0),
                            stop=(c == DC - 1),
                        )
                    t1 = wk.tile([P, 512], f32, tag="t1")
                    nc.vector.tensor_tensor(
                        t1[:], pgc[:], sqxm_b[:, s * 512 : (s + 1) * 512], ADD
                    )
                    ob = op.tile([P, 512], f32, tag="ob")
                    nc.scalar.activation(
                        ob[:],
                        t1[:],
                        mybir.ActivationFunctionType.Sqrt,
                        bias=sqxc_sb[:, kt : kt + 1],
                    )
                    nc.gpsimd.dma_start(
                        o_d.ap()[kt * P : (kt + 1) * P, s * 512 : (s + 1) * 512],
                        ob[:],
                    )

    nc.compile()
    return nc


def _get_nc():
    if "nc" not in _compiled:
        _compiled["nc"] = _build_nc()
    return _compiled["nc"]


def kernel(in_activations, M, centroids):
    from concourse import bass_utils

    X = np.ascontiguousarray(np.asarray(in_activations, dtype=np.float32))
    Mf = np.asarray(M, dtype=np.float32)
    C = np.asarray(centroids, dtype=np.float32)

    nc = _get_nc()

    cst = np.ones((P, P), dtype=np.float32)

    in_maps = []
    for core in range(N_CORES):
        kc, mc = divmod(core, MC)
        ms = np.ascontiguousarray(Mf[:, mc * M_LOC : (mc + 1) * M_LOC])
        cts2 = np.ascontiguousarray(
            (-2.0 * C[kc * K_LOC : (kc + 1) * K_LOC, :].T).astype(np.float32)
        )
        in_maps.append({"x": X, "ms": ms, "cts2": cts2, "cst": cst})

    res = bass_utils.run_bass_kernel_spmd(
        nc,
        in_maps,
        core_ids=list(range(N_CORES)),
        trace=bool(int(os.environ.get("KERNEL_TRACE", "0"))),
    )
    if res.exec_time_ns is not None:
        print(f"HW exec time: {res.exec_time_ns} ns")
        _compiled["exec_time_ns"] = res.exec_time_ns

    out = np.empty((K, M_COLS), dtype=np.float32)
    for core in range(N_CORES):
        kc, mc = divmod(core, MC)
        out[kc * K_LOC : (kc + 1) * K_LOC, mc * M_LOC : (mc + 1) * M_LOC] = res.results[
            core
        ]["out"]
    return out


# revision 10
# speedup vs baseline: 1.4027x; 1.3040x over previous
"""Trainium2 Bass kernel for nn_ComputeDistances (vq_codebook).

dist[k, m] = || X @ (M[:, m] - c_k) ||_2,  X:[4096,512], M:[512,4096], C:[2048,512]

Reformulated via the Gram matrix G = X^T X (512x512):
    dist^2[k, m] = m^T G m  -  2 c_k^T G m  +  c_k^T G c_k
which drops total FLOPs from ~95G to ~14G.

Sharding: 8 cores as a 2(K) x 4(m) grid; each core computes its
[1024, 1024] output slab independently (no collectives).

The whole pipeline runs in fp16 (11-bit mantissa, full PE rate, and -
unlike fp32r - no DMA-produced-operand restriction, so intermediates are
cast on DVE writes with zero DMA traffic). All accumulation is fp32 in
PSUM. Elementwise products that could overflow fp16 are pre-scaled
(M/4, CT2/16) and compensated when the sums are copied out.

Stages per core:
  warmup: tiny matmuls on zero tiles so the PE HAM clock reaches 2.4 GHz
  A:  GXX = X^T X        upper-tri blocks + PE-transpose mirror
  B:  H   = GXX @ Ms     ; sqXM = ones^T (H .* M/4) * 4   (row, bcast)
  B2: GC2 = GXX @ (-2Cs^T); sqXC = ones^T (GC2 .* CT2/16) * 4 (row)
      sqXC column form via PE transpose of the replicated-row blocks
  C:  G2  = (-2Cs^T)^T @ H ; out = sqrt(G2 + sqXM + sqXC)  (DVE + ACT)
"""

import os
import numpy as np

N, D, M_COLS, K = 4096, 512, 4096, 2048
N_CORES = 8
KC, MC = 2, 4  # core grid: K-split x M-split
K_LOC, M_LOC = K // KC, M_COLS // MC  # 1024, 1024

P = 128
NT = N // P        # 32 X row-tiles
DC = D // P        # 4 contraction chunks over D
MS = M_LOC // 512  # 2 m-slices of 512
KS = K_LOC // 512  # 2 k-slices of 512
KT = K_LOC // P    # 8 k-tiles
WARM_MMS = 56

_compiled = {}


def _build_nc():
    import concourse.mybir as mybir
    import concourse.tile as tile
    from concourse import bacc
    from concourse.masks import make_identity

    f32 = mybir.dt.float32
    f16 = mybir.dt.float16
    bf16 = mybir.dt.bfloat16
    ADD = mybir.AluOpType.add
    MULT = mybir.AluOpType.mult

    nc = bacc.Bacc("TRN2", target_bir_lowering=False, debug=False)

    x_d = nc.dram_tensor("x", [N, D], f16, kind="ExternalInput")
    m_d = nc.dram_tensor("ms", [D, M_LOC], f16, kind="ExternalInput")
    c_d = nc.dram_tensor("cts2", [D, K_LOC], f16, kind="ExternalInput")  # -2*C_s^T
    o_d = nc.dram_tensor("out", [K_LOC, M_LOC], f32, kind="ExternalOutput")

    with tile.TileContext(nc) as tc:
        with (
            tc.tile_pool(name="xp", bufs=1) as xp,
            tc.tile_pool(name="inp", bufs=1) as inp,
            tc.tile_pool(name="res", bufs=1) as res,
            tc.tile_pool(name="wk", bufs=2) as wk,
            tc.tile_pool(name="op", bufs=3) as op,
            tc.tile_pool(name="psA", bufs=3, space="PSUM") as psA,
            tc.tile_pool(name="psG", bufs=1, space="PSUM") as psG,
            tc.tile_pool(name="psS", bufs=1, space="PSUM") as psS,
        ):
            # ---- PE warmup: tiny bf16 matmuls on zero tiles (no input deps) ----
            wl = res.tile([P, 1], bf16, tag="wl")
            wz = res.tile([P, 64], bf16, tag="wz")
            nc.vector.memset(wl[:], 0.0)
            nc.vector.memset(wz[:], 0.0)
            wps = psG.tile([1, 64], mybir.dt.float32, tag="gxx")
            for _ in range(WARM_MMS):
                nc.tensor.matmul(wps[:], wl[:], wz[:], start=True, stop=True)

            # ---- input loads (split across the two HWDGE queues) ----
            xt = []
            dma_engs = [nc.sync, nc.scalar]
            for i in range(NT):
                t = xp.tile([P, D], f16, tag=f"x{i}", name=f"x{i}")
                dma_engs[i % 2].dma_start(t[:], x_d.ap()[i * P : (i + 1) * P, :])
                xt.append(t)
            ms16, ct16 = [], []
            for c in range(DC):
                t = inp.tile([P, M_LOC], f16, tag=f"ms{c}", name=f"ms{c}")
                nc.sync.dma_start(t[:], m_d.ap()[c * P : (c + 1) * P, :])
                ms16.append(t)
                t = inp.tile([P, K_LOC], f16, tag=f"ct{c}", name=f"ct{c}")
                nc.scalar.dma_start(t[:], c_d.ap()[c * P : (c + 1) * P, :])
                ct16.append(t)

            ones16 = res.tile([P, P], f16, tag="ones16")
            nc.vector.memset(ones16[:], 1.0)
            ident = res.tile([P, P], f16, tag="ident")
            make_identity(nc, ident[:])
            identf = res.tile([P, P], f32, tag="identf")
            make_identity(nc, identf[:])

            # device-side scaled copies for overflow-safe elementwise products
            msq = [
                res.tile([P, M_LOC], f16, tag=f"msq{c}", name=f"msq{c}")
                for c in range(DC)
            ]
            ct16th = [
                res.tile([P, K_LOC], f16, tag=f"ct16th{c}", name=f"ct16th{c}")
                for c in range(DC)
            ]
            for c in range(DC):
                nc.vector.tensor_scalar_mul(msq[c][:], ms16[c][:], 0.25)
                nc.vector.tensor_scalar_mul(ct16th[c][:], ct16[c][:], 0.0625)

            # resident intermediates
            gxx16 = [
                res.tile([P, D], f16, tag=f"gxx{t}", name=f"gxx{t}") for t in range(DC)
            ]
            hf16 = [
                res.tile([P, M_LOC], f16, tag=f"hf{t}", name=f"hf{t}")
                for t in range(DC)
            ]
            sqxm_b = res.tile([P, M_LOC], f32, tag="sqxm_b")
            sqxc_row = res.tile([P, K_LOC], f32, tag="sqxc_row")
            sqxc_sb = res.tile([P, KT], f32, tag="sqxc_sb")

            # ---- stage A: GXX = X^T X (upper-triangular blocks + mirror) ----
            for t in range(DC):
                width = D - t * P
                pg = psG.tile([P, D], mybir.dt.float32, tag="gxx")
                for i in range(NT):
                    nc.tensor.matmul(
                        pg[:, :width],
                        xt[i][:, t * P : (t + 1) * P],
                        xt[i][:, t * P :],
                        start=(i == 0),
                        stop=(i == NT - 1),
                    )
                nc.vector.tensor_copy(gxx16[t][:, t * P :], pg[:, :width])
                for c in range(t + 1, DC):
                    tp = psA.tile([P, 512], f16, tag="ph")
                    nc.tensor.transpose(
                        tp[:, :P], gxx16[t][:, c * P : (c + 1) * P], ident[:]
                    )
                    nc.vector.tensor_copy(gxx16[c][:, t * P : (t + 1) * P], tp[:, :P])

            # ---- stage B: H = GXX @ Ms ; sqXM via ones-block matmul ----
            # ones-block stationary [128,128] => every PSUM partition gets the
            # same column sum, i.e. sqXM arrives already partition-broadcast.
            sqm = [
                psS.tile([P, 512], mybir.dt.float32, tag=f"sqm{s}", name=f"sqm{s}")
                for s in range(MS)
            ]
            for t in range(DC):
                for s in range(MS):
                    ph = psA.tile([P, 512], mybir.dt.float32, tag="ph")
                    for c in range(DC):
                        nc.tensor.matmul(
                            ph[:],
                            gxx16[c][:, t * P : (t + 1) * P],
                            ms16[c][:, s * 512 : (s + 1) * 512],
                            start=(c == 0),
                            stop=(c == DC - 1),
                        )
                    nc.vector.tensor_copy(hf16[t][:, s * 512 : (s + 1) * 512], ph[:])
                p16 = wk.tile([P, M_LOC], f16, tag="p16")
                nc.vector.tensor_tensor(p16[:], hf16[t][:], msq[t][:], MULT)
                for s in range(MS):
                    nc.tensor.matmul(
                        sqm[s][:],
                        ones16[:],
                        p16[:, s * 512 : (s + 1) * 512],
                        start=(t == 0),
                        stop=(t == DC - 1),
                    )
                if t == DC - 1:
                    for s in range(MS):
                        nc.vector.tensor_scalar_mul(
                            sqxm_b[:, s * 512 : (s + 1) * 512], sqm[s][:], 4.0
                        )

            # ---- stage B2: GC2 = GXX @ (-2 CTs) ; sqXC via ones-block matmul ----
            sqc = [
                psS.tile([P, 512], mybir.dt.float32, tag=f"sqc{s}", name=f"sqc{s}")
                for s in range(KS)
            ]
            for t in range(DC):
                gcf = wk.tile([P, K_LOC], f32, tag="gcf")
                for s in range(KS):
                    ph = psA.tile([P, 512], mybir.dt.float32, tag="ph")
                    for c in range(DC):
                        nc.tensor.matmul(
                            ph[:],
                            gxx16[c][:, t * P : (t + 1) * P],
                            ct16[c][:, s * 512 : (s + 1) * 512],
                            start=(c == 0),
                            stop=(c == DC - 1),
                        )
                    nc.vector.tensor_copy(gcf[:, s * 512 : (s + 1) * 512], ph[:])
                q16 = wk.tile([P, K_LOC], f16, tag="q16")
                nc.vector.tensor_tensor(q16[:], gcf[:], ct16th[t][:], MULT)
                for s in range(KS):
                    nc.tensor.matmul(
                        sqc[s][:],
                        ones16[:],
                        q16[:, s * 512 : (s + 1) * 512],
                        start=(t == 0),
                        stop=(t == DC - 1),
                    )
                if t == DC - 1:
                    for s in range(KS):
                        nc.vector.tensor_scalar_mul(
                            sqxc_row[:, s * 512 : (s + 1) * 512], sqc[s][:], 4.0
                        )
                    # extract column form: transpose each replicated-row block;
                    # column 0 then holds sqXC for that k-tile
                    for kt in range(KT):
                        tpc = psA.tile([P, 512], mybir.dt.float32, tag="ph")
                        nc.tensor.transpose(
                            tpc[:, :P],
                            sqxc_row[:, kt * P : (kt + 1) * P],
                            identf[:],
                        )
                        nc.vector.tensor_copy(sqxc_sb[:, kt : kt + 1], tpc[:, 0:1])

            # ---- stage C: G2 = (-2CTs)^T @ H ; combine ; sqrt ----
            for kt in range(KT):
                for s in range(MS):
                    pgc = psA.tile([P, 512], mybir.dt.float32, tag="ph")
                    for c in range(DC):
                        nc.tensor.matmul(
                            pgc[:],
                            ct16[c][:, kt * P : (kt + 1) * P],
                            hf16[c][:, s * 512 : (s + 1) * 512],
                            start=(c == 0),
                            stop=(c == DC - 1),
                        )
                    t1 = wk.tile([P, 512], f32, tag="t1")
                    nc.vector.tensor_tensor(
                        t1[:], pgc[:], sqxm_b[:, s * 512 : (s + 1) * 512], ADD
                    )
                    ob = op.tile([P, 512], f32, tag="ob")
                    nc.scalar.activation(
                        ob[:],
                        t1[:],
                        mybir.ActivationFunctionType.Sqrt,
                        bias=sqxc_sb[:, kt : kt + 1],
                    )
                    (nc.sync if (kt + s) % 2 == 0 else nc.gpsimd).dma_start(
                        o_d.ap()[kt * P : (kt + 1) * P, s * 512 : (s + 1) * 512],
                        ob[:],
                    )

    nc.compile()
    return nc


def _get_nc():
    if "nc" not in _compiled:
        _compiled["nc"] = _build_nc()
    return _compiled["nc"]


def kernel(in_activations, M, centroids):
    from concourse import bass_utils

    X = np.asarray(in_activations, dtype=np.float32)
    Mf = np.asarray(M, dtype=np.float32)
    C = np.asarray(centroids, dtype=np.float32)

    nc = _get_nc()

    x16 = np.ascontiguousarray(X.astype(np.float16))
    in_maps = []
    for core in range(N_CORES):
        kc, mc = divmod(core, MC)
        ms = np.ascontiguousarray(
            Mf[:, mc * M_LOC : (mc + 1) * M_LOC].astype(np.float16)
        )
        cts2 = np.ascontiguousarray(
            (-2.0 * C[kc * K_LOC : (kc + 1) * K_LOC, :].T).astype(np.float16)
        )
        in_maps.append({"x": x16, "ms": ms, "cts2": cts2})

    res = bass_utils.run_bass_kernel_spmd(
        nc,
        in_maps,
        core_ids=list(range(N_CORES)),
        trace=bool(int(os.environ.get("KERNEL_TRACE", "0"))),
    )
    if res.exec_time_ns is not None:
        print(f"HW exec time: {res.exec_time_ns} ns")
        _compiled["exec_time_ns"] = res.exec_time_ns

    out = np.empty((K, M_COLS), dtype=np.float32)
    for core in range(N_CORES):
        kc, mc = divmod(core, MC)
        out[kc * K_LOC : (kc + 1) * K_LOC, mc * M_LOC : (mc + 1) * M_LOC] = res.results[
            core
        ]["out"]
    return out


# revision 11
# speedup vs baseline: 1.5124x; 1.0782x over previous
"""Trainium2 Bass kernel for nn_ComputeDistances (vq_codebook).

dist[k, m] = || X @ (M[:, m] - c_k) ||_2,  X:[4096,512], M:[512,4096], C:[2048,512]

Reformulated via the Gram matrix G = X^T X (512x512):
    dist^2[k, m] = m^T G m  -  2 c_k^T G m  +  c_k^T G c_k
which drops total FLOPs from ~95G to ~14G.

Sharding: 8 cores as a 2(K) x 4(m) grid; each core computes its
[1024, 1024] output slab independently (no collectives).

The whole pipeline runs in fp16 (11-bit mantissa, full PE rate, and -
unlike fp32r - no DMA-produced-operand restriction, so intermediates are
cast on DVE writes with zero DMA traffic). All accumulation is fp32 in
PSUM. Elementwise products that could overflow fp16 are pre-scaled
(M/4, CT2/16) and compensated when the sums are copied out.

Stages per core:
  warmup: tiny matmuls on zero tiles so the PE HAM clock reaches 2.4 GHz
  A:  GXX = X^T X        upper-tri blocks + PE-transpose mirror
  B:  H   = GXX @ Ms     ; sqXM = ones^T (H .* M/4) * 4   (row, bcast)
  B2: GC2 = GXX @ (-2Cs^T); sqXC = ones^T (GC2 .* CT2/16) * 4 (row)
      sqXC column form via PE transpose of the replicated-row blocks
  C:  G2  = (-2Cs^T)^T @ H ; out = sqrt(G2 + sqXM + sqXC)  (DVE + ACT)
"""

import os
import numpy as np

N, D, M_COLS, K = 4096, 512, 4096, 2048
N_CORES = 8
KC, MC = 2, 4  # core grid: K-split x M-split
K_LOC, M_LOC = K // KC, M_COLS // MC  # 1024, 1024

P = 128
NT = N // P        # 32 X row-tiles
DC = D // P        # 4 contraction chunks over D
MS = M_LOC // 512  # 2 m-slices of 512
KS = K_LOC // 512  # 2 k-slices of 512
KT = K_LOC // P    # 8 k-tiles
WARM_MMS = 56

_compiled = {}


def _build_nc():
    import concourse.mybir as mybir
    import concourse.tile as tile
    from concourse import bacc
    from concourse.masks import make_identity

    f32 = mybir.dt.float32
    f16 = mybir.dt.float16
    bf16 = mybir.dt.bfloat16
    ADD = mybir.AluOpType.add
    MULT = mybir.AluOpType.mult

    nc = bacc.Bacc("TRN2", target_bir_lowering=False, debug=False)

    x_d = nc.dram_tensor("x", [N, D], f16, kind="ExternalInput")
    m_d = nc.dram_tensor("ms", [D, M_LOC], f16, kind="ExternalInput")
    c_d = nc.dram_tensor("cts2", [D, K_LOC], f16, kind="ExternalInput")  # -2*C_s^T
    o_d = nc.dram_tensor("out", [K_LOC, M_LOC], f32, kind="ExternalOutput")

    with tile.TileContext(nc) as tc:
        with (
            tc.tile_pool(name="xp", bufs=1) as xp,
            tc.tile_pool(name="inp", bufs=1) as inp,
            tc.tile_pool(name="res", bufs=1) as res,
            tc.tile_pool(name="wk", bufs=2) as wk,
            tc.tile_pool(name="op", bufs=3) as op,
            tc.tile_pool(name="psA", bufs=3, space="PSUM") as psA,
            tc.tile_pool(name="psG", bufs=1, space="PSUM") as psG,
            tc.tile_pool(name="psS", bufs=1, space="PSUM") as psS,
        ):
            # ---- PE warmup: tiny bf16 matmuls on zero tiles (no input deps) ----
            wl = res.tile([P, 1], bf16, tag="wl")
            wz = res.tile([P, 64], bf16, tag="wz")
            nc.vector.memset(wl[:], 0.0)
            nc.vector.memset(wz[:], 0.0)
            wps = psG.tile([1, 64], mybir.dt.float32, tag="gxx")
            for _ in range(WARM_MMS):
                nc.tensor.matmul(wps[:], wl[:], wz[:], start=True, stop=True)

            # ---- input loads (split across the two HWDGE queues) ----
            xq = []
            dma_engs = [nc.sync, nc.scalar]
            NQ = N // (P * 4)  # 8 big X tiles, 4 rows per partition
            for j in range(NQ):
                t = xp.tile([P, 4, D], f16, tag=f"xq{j}", name=f"xq{j}")
                src_ap = x_d.ap()[j * 4 * P : (j + 1) * 4 * P, :].rearrange(
                    "(p four) d -> p four d", four=4
                )
                dma_engs[j % 2].dma_start(t[:], src_ap)
                xq.append(t)
            ms16, ct16 = [], []
            for c in range(DC):
                t = inp.tile([P, M_LOC], f16, tag=f"ms{c}", name=f"ms{c}")
                nc.sync.dma_start(t[:], m_d.ap()[c * P : (c + 1) * P, :])
                ms16.append(t)
                t = inp.tile([P, K_LOC], f16, tag=f"ct{c}", name=f"ct{c}")
                nc.scalar.dma_start(t[:], c_d.ap()[c * P : (c + 1) * P, :])
                ct16.append(t)

            ones16 = res.tile([P, P], f16, tag="ones16")
            nc.vector.memset(ones16[:], 1.0)
            ident = res.tile([P, P], f16, tag="ident")
            make_identity(nc, ident[:])
            identf = res.tile([P, P], f32, tag="identf")
            make_identity(nc, identf[:])

            # device-side scaled copies for overflow-safe elementwise products
            msq = [
                res.tile([P, M_LOC], f16, tag=f"msq{c}", name=f"msq{c}")
                for c in range(DC)
            ]
            ct16th = [
                res.tile([P, K_LOC], f16, tag=f"ct16th{c}", name=f"ct16th{c}")
                for c in range(DC)
            ]
            for c in range(DC):
                nc.vector.tensor_scalar_mul(msq[c][:], ms16[c][:], 0.25)
                nc.vector.tensor_scalar_mul(ct16th[c][:], ct16[c][:], 0.0625)

            # resident intermediates
            gxx16 = [
                res.tile([P, D], f16, tag=f"gxx{t}", name=f"gxx{t}") for t in range(DC)
            ]
            hf16 = [
                res.tile([P, M_LOC], f16, tag=f"hf{t}", name=f"hf{t}")
                for t in range(DC)
            ]
            sqxm_b = res.tile([P, M_LOC], f32, tag="sqxm_b")
            sqxc_row = res.tile([P, K_LOC], f32, tag="sqxc_row")
            sqxc_sb = res.tile([P, KT], f32, tag="sqxc_sb")

            # ---- stage A: GXX = X^T X (upper-triangular blocks + mirror) ----
            for t in range(DC):
                width = D - t * P
                pg = psG.tile([P, D], mybir.dt.float32, tag="gxx")
                NQ = N // (P * 4)
                for i in range(NT):
                    j, r = divmod(i, 4)
                    nc.tensor.matmul(
                        pg[:, :width],
                        xq[j][:, r, t * P : (t + 1) * P],
                        xq[j][:, r, t * P :],
                        start=(i == 0),
                        stop=(i == NT - 1),
                    )
                nc.vector.tensor_copy(gxx16[t][:, t * P :], pg[:, :width])
                for c in range(t + 1, DC):
                    tp = psA.tile([P, 512], f16, tag="ph")
                    nc.tensor.transpose(
                        tp[:, :P], gxx16[t][:, c * P : (c + 1) * P], ident[:]
                    )
                    nc.vector.tensor_copy(gxx16[c][:, t * P : (t + 1) * P], tp[:, :P])

            # ---- stage B: H = GXX @ Ms ; sqXM via ones-block matmul ----
            # ones-block stationary [128,128] => every PSUM partition gets the
            # same column sum, i.e. sqXM arrives already partition-broadcast.
            sqm = [
                psS.tile([P, 512], mybir.dt.float32, tag=f"sqm{s}", name=f"sqm{s}")
                for s in range(MS)
            ]
            for t in range(DC):
                for s in range(MS):
                    ph = psA.tile([P, 512], mybir.dt.float32, tag="ph")
                    for c in range(DC):
                        nc.tensor.matmul(
                            ph[:],
                            gxx16[c][:, t * P : (t + 1) * P],
                            ms16[c][:, s * 512 : (s + 1) * 512],
                            start=(c == 0),
                            stop=(c == DC - 1),
                        )
                    nc.vector.tensor_copy(hf16[t][:, s * 512 : (s + 1) * 512], ph[:])
                p16 = wk.tile([P, M_LOC], f16, tag="p16")
                nc.vector.tensor_tensor(p16[:], hf16[t][:], msq[t][:], MULT)
                for s in range(MS):
                    nc.tensor.matmul(
                        sqm[s][:],
                        ones16[:],
                        p16[:, s * 512 : (s + 1) * 512],
                        start=(t == 0),
                        stop=(t == DC - 1),
                    )
                if t == DC - 1:
                    for s in range(MS):
                        nc.vector.tensor_scalar_mul(
                            sqxm_b[:, s * 512 : (s + 1) * 512], sqm[s][:], 4.0
                        )

            # ---- stage B2: GC2 = GXX @ (-2 CTs) ; sqXC via ones-block matmul ----
            sqc = [
                psS.tile([P, 512], mybir.dt.float32, tag=f"sqc{s}", name=f"sqc{s}")
                for s in range(KS)
            ]
            for t in range(DC):
                q16 = wk.tile([P, K_LOC], f16, tag="q16")
                for s in range(KS):
                    ph = psA.tile([P, 512], mybir.dt.float32, tag="ph")
                    for c in range(DC):
                        nc.tensor.matmul(
                            ph[:],
                            gxx16[c][:, t * P : (t + 1) * P],
                            ct16[c][:, s * 512 : (s + 1) * 512],
                            start=(c == 0),
                            stop=(c == DC - 1),
                        )
                    nc.vector.tensor_tensor(
                        q16[:, s * 512 : (s + 1) * 512],
                        ph[:],
                        ct16th[t][:, s * 512 : (s + 1) * 512],
                        MULT,
                    )
                for s in range(KS):
                    nc.tensor.matmul(
                        sqc[s][:],
                        ones16[:],
                        q16[:, s * 512 : (s + 1) * 512],
                        start=(t == 0),
                        stop=(t == DC - 1),
                    )
                if t == DC - 1:
                    for s in range(KS):
                        nc.vector.tensor_scalar_mul(
                            sqxc_row[:, s * 512 : (s + 1) * 512], sqc[s][:], 4.0
                        )
                    # extract column form: transpose each replicated-row block;
                    # column 0 then holds sqXC for that k-tile
                    for kt in range(KT):
                        tpc = psA.tile([P, 512], mybir.dt.float32, tag="ph")
                        nc.tensor.transpose(
                            tpc[:, :P],
                            sqxc_row[:, kt * P : (kt + 1) * P],
                            identf[:],
                        )
                        nc.vector.tensor_copy(sqxc_sb[:, kt : kt + 1], tpc[:, 0:1])

            # ---- stage C: G2 = (-2CTs)^T @ H ; combine ; sqrt ----
            for kt in range(KT):
                for s in range(MS):
                    pgc = psA.tile([P, 512], mybir.dt.float32, tag="ph")
                    for c in range(DC):
                        nc.tensor.matmul(
                            pgc[:],
                            ct16[c][:, kt * P : (kt + 1) * P],
                            hf16[c][:, s * 512 : (s + 1) * 512],
                            start=(c == 0),
                            stop=(c == DC - 1),
                        )
                    t1 = wk.tile([P, 512], f32, tag="t1")
                    nc.vector.tensor_tensor(
                        t1[:], pgc[:], sqxm_b[:, s * 512 : (s + 1) * 512], ADD
                    )
                    ob = op.tile([P, 512], f32, tag="ob")
                    nc.scalar.activation(
                        ob[:],
                        t1[:],
                        mybir.ActivationFunctionType.Sqrt,
                        bias=sqxc_sb[:, kt : kt + 1],
                    )
                    (nc.sync if (kt + s) % 2 == 0 else nc.gpsimd).dma_start(
                        o_d.ap()[kt * P : (kt + 1) * P, s * 512 : (s + 1) * 512],
                        ob[:],
                    )

    nc.compile()
    return nc


def _get_nc():
    if "nc" not in _compiled:
        _compiled["nc"] = _build_nc()
    return _compiled["nc"]


def kernel(in_activations, M, centroids):
    from concourse import bass_utils

    X = np.asarray(in_activations, dtype=np.float32)
    Mf = np.asarray(M, dtype=np.float32)
    C = np.asarray(centroids, dtype=np.float32)

    nc = _get_nc()

    x16 = np.ascontiguousarray(X.astype(np.float16))
    in_maps = []
    for core in range(N_CORES):
        kc, mc = divmod(core, MC)
        ms = np.ascontiguousarray(
            Mf[:, mc * M_LOC : (mc + 1) * M_LOC].astype(np.float16)
        )
        cts2 = np.ascontiguousarray(
            (-2.0 * C[kc * K_LOC : (kc + 1) * K_LOC, :].T).astype(np.float16)
        )
        in_maps.append({"x": x16, "ms": ms, "cts2": cts2})

    res = bass_utils.run_bass_kernel_spmd(
        nc,
        in_maps,
        core_ids=list(range(N_CORES)),
        trace=bool(int(os.environ.get("KERNEL_TRACE", "0"))),
    )
    if res.exec_time_ns is not None:
        print(f"HW exec time: {res.exec_time_ns} ns")
        _compiled["exec_time_ns"] = res.exec_time_ns

    out = np.empty((K, M_COLS), dtype=np.float32)
    for core in range(N_CORES):
        kc, mc = divmod(core, MC)
        out[kc * K_LOC : (kc + 1) * K_LOC, mc * M_LOC : (mc + 1) * M_LOC] = res.results[
            core
        ]["out"]
    return out
